# revision 8
# baseline (speedup 1.0000x reference)
"""AttentionGAT (2-layer GAT + attention fusion gate + mean-pool + MLP head)
as a Bass/Tile kernel on 8 Trainium2 NeuronCores.

Strategy:
  Phase 1 (node-parallel): each core computes h1 = fused @ W1_aug for its
    3840-node shard. The fusion gate softmax(x @ attn_W) is obtained by
    splitting the K-contraction at the hog/cov boundary (h1 = g0*A + g1*B)
    and folding attn_W / alpha-projection vectors in as extra N columns.
  AllGather h_aug (h1 + alpha_src + alpha_dst per node, 768B bf16 rows).
  Phase 2a (dst-parallel): edges sorted by destination, grouped per
    128-dst-node block.  dma_gather fetches h_aug[src] rows; a second small
    gather fetches alpha_dst[dst].  e = exp(leakyrelu(as+ad)) is computed in
    a bulk [128, T] layout; per 128-edge tile a fused DVE op builds
    S_alpha = (iota == dst_local) * e and the TensorEngine accumulates
    S_alpha^T @ h (plus a ones column for the softmax denominator) in PSUM.
    Normalization by the denominator happens once per dst node after
    aggregation (softmax max-subtraction is skipped: the logits are O(1)).
  AllGather h2_aug; Phase 2b repeats the aggregation for layer 2 (1 head)
  and accumulates the graph mean-pool partials with a batch-onehot matmul.
  The [64,128] per-core partials are summed, normalized and pushed through
  the tiny classifier on the host.
"""

import os
import sys
import types

sys.path.insert(0, "/opt/trn_rl_repo")

import numpy as np
import ml_dtypes

import concourse.bass as bass
import concourse.mybir as mybir
import concourse.tile as tile
from concourse import bacc
from concourse.bass_utils import run_bass_kernel_spmd
from concourse.masks import make_identity

BF16 = ml_dtypes.bfloat16
NCORES = 8
LAST_EXEC_NS = None  # set when AGAT_PROFILE=1


def _install_ntff_hook():
    """Shim antenv.axon_hooks so run_bass_kernel_spmd(trace=True) can
    capture NTFF profiles through the axon PJRT plugin."""
    try:
        from antenv.axon_hooks import get_axon_ntff_profile_hook  # noqa: F401
        return
    except ImportError:
        pass
    try:
        import antenv
        from trn_agent_boot.trn_boot import _ntff_profile_via_ctypes

        mod = types.ModuleType("antenv.axon_hooks")
        _h = [None]
        mod.set_axon_ntff_profile_hook = lambda h: _h.__setitem__(0, h)
        mod.get_axon_ntff_profile_hook = lambda: _h[0]
        sys.modules["antenv.axon_hooks"] = mod
        antenv.axon_hooks = mod
        mod.set_axon_ntff_profile_hook(
            _ntff_profile_via_ctypes("/opt/axon/libaxon_pjrt.so")
        )
    except Exception:
        pass


def _wrap_idx(a):
    """dma_gather index layout: idx i at [i%16, i//16], tiled to 128 parts."""
    return np.tile(a.reshape(-1, 16).T, (8, 1)).astype(np.int16)


def _ktiles(in_dim, hog):
    """K-tile boundaries of <=128, with a cut exactly at `hog`; the tiles at
    or past `hog` form the cov part (must be a single tile)."""
    kt = []
    s = 0
    while s < hog:
        e = min(s + 128, hog)
        kt.append((s, e))
        s = e
    assert in_dim - hog <= 128
    kt.append((hog, in_dim))
    return kt


def build_program(cfg):
    IN_DIM, HOG = cfg["in_dim"], cfg["hog"]
    NPC, NBLK, NT1, NG = cfg["npc"], cfg["nblk"], cfg["nt1"], cfg["ng"]
    NPAD = NPC * NCORES
    KT = _ktiles(IN_DIM, HOG)
    NKT = len(KT)
    NC1 = 262          # 256 h cols + as0 as1 ad0 ad1 + gate0 gate1
    ROW1, ROW2 = 260, 130
    MG = 2             # node-tiles per phase-1 PSUM group
    NM = NPC // 128

    dt = mybir.dt
    nc = bacc.Bacc("TRN2", target_bir_lowering=False, debug=False,
                   num_devices=NCORES)

    xT = nc.dram_tensor("xT", [IN_DIM, NPC], dt.bfloat16, kind="ExternalInput").ap()
    w1 = nc.dram_tensor("w1", [IN_DIM, NC1], dt.bfloat16, kind="ExternalInput").ap()
    w2 = nc.dram_tensor("w2", [2, 128, 132], dt.bfloat16, kind="ExternalInput").ap()
    b1b = nc.dram_tensor("b1b", [128, 256], dt.float32, kind="ExternalInput").ap()
    b2b = nc.dram_tensor("b2b", [128, 128], dt.float32, kind="ExternalInput").ap()
    abb = nc.dram_tensor("abb", [128, 2], dt.float32, kind="ExternalInput").ap()
    iota = nc.dram_tensor("iota", [128, 128], dt.bfloat16, kind="ExternalInput").ap()
    sidx = nc.dram_tensor("sidx", [NBLK, 128, NT1], dt.int32, kind="ExternalInput").ap()
    onehotT = nc.dram_tensor("onehotT", [NBLK, 128, NT1 * 128], dt.bfloat16, kind="ExternalInput").ap()
    dloc = nc.dram_tensor("dloc", [NBLK, 128, NT1], dt.float32, kind="ExternalInput").ap()
    bhot = nc.dram_tensor("bhot", [NBLK, 128, NG], dt.bfloat16, kind="ExternalInput").ap()
    pout = nc.dram_tensor("pout", [NG, 128], dt.float32, kind="ExternalOutput").ap()

    AOT = mybir.AluOpType
    AFT = mybir.ActivationFunctionType

    with tile.TileContext(nc) as tc:
        with (
            tc.tile_pool(name="constp", bufs=1) as constp,
            tc.tile_pool(name="dramp", bufs=1, space="DRAM") as dramp,
        ):
            haug_sh = dramp.tile([NPC, ROW1], dt.bfloat16)
            haug = dramp.tile([NPAD, ROW1], dt.bfloat16, addr_space="Shared")
            h2_sh = dramp.tile([NPC, ROW2], dt.bfloat16)
            h2f = dramp.tile([NPAD, ROW2], dt.bfloat16, addr_space="Shared")

            iota_sb = constp.tile([128, 128], dt.bfloat16)
            nc.sync.dma_start(iota_sb[:], iota[:])
            ones_sb = constp.tile([128, 1], dt.bfloat16)
            nc.vector.memset(ones_sb[:], 1.0)
            ident = constp.tile([128, 128], dt.bfloat16)
            make_identity(nc, ident[:])
            b1_sb = constp.tile([128, 256], dt.float32)
            nc.sync.dma_start(b1_sb[:], b1b[:])
            b2_sb = constp.tile([128, 128], dt.float32)
            nc.sync.dma_start(b2_sb[:], b2b[:])
            ab_sb = constp.tile([128, 2], dt.float32)
            nc.sync.dma_start(ab_sb[:], abb[:])
            w2_sb = []
            for kk in range(2):
                t = constp.tile([128, 132], dt.bfloat16, tag=f"w2_{kk}",
                                name=f"w2sb{kk}")
                nc.sync.dma_start(t[:], w2[kk])
                w2_sb.append(t)
            w1_sb = []
            for k, (k0, k1) in enumerate(KT):
                t = constp.tile([k1 - k0, NC1], dt.bfloat16, tag=f"w1_{k}",
                                name=f"w1sb{k}")
                nc.sync.dma_start(t[:], w1[k0:k1, :])
                w1_sb.append(t)

            # ---------------- phase 1: h_aug for own node shard ------------
            with (
                tc.tile_pool(name="p1", bufs=3) as p1,
                tc.tile_pool(name="p1o", bufs=2) as p1o,
                tc.tile_pool(name="p1ps", bufs=2, space="PSUM") as p1ps,
            ):
                for g0m in range(0, NM, MG):
                    ms = list(range(g0m, min(g0m + MG, NM)))
                    psA = [p1ps.tile([128, NC1], dt.float32, tag=f"A{i}",
                                     name=f"psA{g0m}_{i}") for i in range(len(ms))]
                    psB = [p1ps.tile([128, NC1], dt.float32, tag=f"B{i}",
                                     name=f"psB{g0m}_{i}") for i in range(len(ms))]
                    for k, (k0, k1) in enumerate(KT):
                        slab = p1.tile([k1 - k0, len(ms) * 128], dt.bfloat16,
                                       tag="slab", name=f"slab{g0m}_{k}")
                        nc.sync.dma_start(
                            slab[:], xT[k0:k1, ms[0] * 128:(ms[-1] + 1) * 128])
                        cov = (k == NKT - 1)
                        for i in range(len(ms)):
                            nc.tensor.matmul(
                                (psB[i] if cov else psA[i])[:],
                                slab[:, i * 128:(i + 1) * 128],
                                w1_sb[k][:],
                                start=(k == 0 or cov),
                                stop=(k == NKT - 2 or cov),
                            )
                    for i, m in enumerate(ms):
                        lg = p1o.tile([128, 2], dt.float32, tag="lg", name=f"lg{m}")
                        nc.vector.tensor_copy(lg[:], psA[i][:, 260:262])
                        nc.vector.tensor_tensor(lg[:], lg[:],
                                                psB[i][:, 260:262], AOT.add)
                        nc.vector.tensor_tensor(lg[:], lg[:], ab_sb[:], AOT.add)
                        mx = p1o.tile([128, 1], dt.float32, tag="mx", name=f"mx{m}")
                        nc.vector.tensor_reduce(mx[:], lg[:],
                                                mybir.AxisListType.X, AOT.max)
                        mxn = p1o.tile([128, 1], dt.float32, tag="mxn", name=f"mxn{m}")
                        nc.vector.tensor_scalar(mxn[:], mx[:], -1.0, None, AOT.mult)
                        em = p1o.tile([128, 2], dt.float32, tag="em", name=f"em{m}")
                        nc.scalar.activation(em[:], lg[:], AFT.Exp, bias=mxn[:, 0:1])
                        sm = p1o.tile([128, 1], dt.float32, tag="sm", name=f"sm{m}")
                        nc.vector.tensor_reduce(sm[:], em[:],
                                                mybir.AxisListType.X, AOT.add)
                        rs = p1o.tile([128, 1], dt.float32, tag="rs", name=f"rs{m}")
                        nc.vector.reciprocal(rs[:], sm[:])
                        gg = p1o.tile([128, 2], dt.float32, tag="gg", name=f"gg{m}")
                        nc.vector.tensor_scalar(gg[:], em[:], rs[:, 0:1], None,
                                                AOT.mult)
                        h1 = p1o.tile([128, 260], dt.float32, tag="h1", name=f"h1{m}")
                        tmb = p1o.tile([128, 260], dt.float32, tag="tmb", name=f"tmb{m}")
                        nc.vector.tensor_scalar(h1[:], psA[i][:, 0:260],
                                                gg[:, 0:1], None, AOT.mult)
                        nc.vector.tensor_scalar(tmb[:], psB[i][:, 0:260],
                                                gg[:, 1:2], None, AOT.mult)
                        nc.vector.tensor_tensor(h1[:], h1[:], tmb[:], AOT.add)
                        ha = p1o.tile([128, ROW1], dt.bfloat16, tag="ha", name=f"ha{m}")
                        nc.vector.tensor_copy(ha[:, 0:260], h1[:])
                        nc.sync.dma_start(haug_sh[m * 128:(m + 1) * 128, :], ha[:])

            nc.gpsimd.collective_compute(
                "AllGather", AOT.bypass,
                replica_groups=[list(range(NCORES))],
                ins=[haug_sh.opt()], outs=[haug.opt()],
            )

            # ---------------- phase 2a: layer-1 aggregation -> h2_aug ------
            with (
                tc.tile_pool(name="p2g", bufs=3) as p2g,
                tc.tile_pool(name="p2s", bufs=4) as p2s,
                tc.tile_pool(name="p2o", bufs=2) as p2o,
                tc.tile_pool(name="p2ps", bufs=1, space="PSUM") as p2ps,
            ):
                for j in range(NBLK):
                    osb = p2s.tile([128, NT1], dt.int32, tag="osb", name=f"osb{j}")
                    nc.sync.dma_start(osb[:], sidx[j])
                    dl = p2s.tile([128, NT1], dt.float32, tag="dl", name=f"dl{j}")
                    nc.sync.dma_start(dl[:], dloc[j])
                    oT = p2g.tile([128, NT1 * 128], dt.bfloat16, tag="oT",
                                  name=f"oT{j}")
                    nc.sync.dma_start(oT[:], onehotT[j])
                    adt = p2s.tile([128, 2], dt.bfloat16, tag="adt", name=f"adt{j}")
                    nc.sync.dma_start(adt[:],
                                      haug_sh[j * 128:(j + 1) * 128, 258:260])
                    G = p2g.tile([128, NT1, ROW1], dt.bfloat16, tag="G", name=f"G{j}")
                    for t in range(NT1):
                        nc.gpsimd.indirect_dma_start(
                            G[:, t, :], None, haug[:],
                            bass.IndirectOffsetOnAxis(ap=osb[:, t:t + 1], axis=0))
                    adps = p2ps.tile([128, 2 * NT1], dt.float32, tag="adps",
                                     name=f"adps{j}")
                    for t in range(NT1):
                        nc.tensor.matmul(adps[:, 2 * t:2 * t + 2],
                                         oT[:, t * 128:(t + 1) * 128],
                                         adt[:], start=True, stop=True)
                    ade = p2s.tile([128, 2 * NT1], dt.float32, tag="ade",
                                   name=f"ade{j}")
                    nc.vector.tensor_copy(ade[:], adps[:])
                    es = []
                    for h in (0, 1):
                        z = p2s.tile([128, NT1], dt.float32, tag=f"z{h}",
                                     name=f"z{h}_{j}")
                        nc.vector.tensor_tensor(z[:], G[:, :, 256 + h],
                                                ade[:, h::2], AOT.add)
                        z2 = p2s.tile([128, NT1], dt.float32, tag=f"z2{h}",
                                      name=f"z2{h}_{j}")
                        nc.vector.tensor_scalar(z2[:], z[:], 0.2, None, AOT.mult)
                        nc.vector.tensor_tensor(z[:], z[:], z2[:], AOT.max)
                        e = p2s.tile([128, NT1], dt.float32, tag=f"e{h}",
                                     name=f"e{h}_{j}")
                        nc.scalar.activation(e[:], z[:], AFT.Exp)
                        es.append(e)
                    accF = [p2ps.tile([128, 128], dt.float32, tag=f"F{h}",
                                      name=f"F{h}_{j}") for h in (0, 1)]
                    accD = [p2ps.tile([128, 1], dt.float32, tag=f"Dn{h}",
                                      name=f"Dn{h}_{j}") for h in (0, 1)]
                    for t in range(NT1):
                        for h in (0, 1):
                            Sa = p2s.tile([128, 128], dt.bfloat16, tag=f"Sa{h}",
                                          name=f"Sa{h}_{j}_{t}")
                            nc.vector.tensor_scalar(
                                Sa[:], iota_sb[:], dl[:, t:t + 1],
                                es[h][:, t:t + 1], AOT.is_equal, AOT.mult)
                            nc.tensor.matmul(accF[h][:], Sa[:],
                                             G[:, t, h * 128:(h + 1) * 128],
                                             start=(t == 0), stop=(t == NT1 - 1))
                            nc.tensor.matmul(accD[h][:], Sa[:], ones_sb[:],
                                             start=(t == 0), stop=(t == NT1 - 1))
                    hr = p2o.tile([128, 256], dt.float32, tag="hr", name=f"hr{j}")
                    for h in (0, 1):
                        den = p2o.tile([128, 1], dt.float32, tag=f"den{h}",
                                       name=f"den{h}_{j}")
                        nc.vector.tensor_scalar(den[:], accD[h][:], 1e-6, None,
                                                AOT.add)
                        rcp = p2o.tile([128, 1], dt.float32, tag=f"rcp{h}",
                                       name=f"rcp{h}_{j}")
                        nc.vector.reciprocal(rcp[:], den[:])
                        nc.vector.tensor_scalar(hr[:, h * 128:(h + 1) * 128],
                                                accF[h][:], rcp[:, 0:1], None,
                                                AOT.mult)
                    nc.vector.tensor_tensor(hr[:], hr[:], b1_sb[:], AOT.add)
                    hrb = p2o.tile([128, 256], dt.bfloat16, tag="hrb", name=f"hrb{j}")
                    nc.vector.tensor_scalar(hrb[:], hr[:], 0.0, None, AOT.max)
                    h2ps = p2ps.tile([128, 132], dt.float32, tag="h2ps",
                                     name=f"h2ps{j}")
                    for kk in range(2):
                        trp = p2ps.tile([128, 128], dt.bfloat16, tag="trp",
                                        name=f"trp{j}_{kk}")
                        nc.tensor.transpose(trp[:],
                                            hrb[:, kk * 128:(kk + 1) * 128],
                                            ident[:])
                        trs = p2s.tile([128, 128], dt.bfloat16, tag="trs",
                                       name=f"trs{j}_{kk}")
                        nc.vector.tensor_copy(trs[:], trp[:])
                        nc.tensor.matmul(h2ps[:], trs[:], w2_sb[kk][:],
                                         start=(kk == 0), stop=(kk == 1))
                    h2a = p2o.tile([128, ROW2], dt.bfloat16, tag="h2a",
                                   name=f"h2a{j}")
                    nc.vector.tensor_copy(h2a[:, 0:130], h2ps[:, 0:130])
                    nc.sync.dma_start(h2_sh[j * 128:(j + 1) * 128, :], h2a[:])

            nc.gpsimd.collective_compute(
                "AllGather", AOT.bypass,
                replica_groups=[list(range(NCORES))],
                ins=[h2_sh.opt()], outs=[h2f.opt()],
            )

            # ---------------- phase 2b: layer-2 aggregation + pooling ------
            with (
                tc.tile_pool(name="p3g", bufs=3) as p3g,
                tc.tile_pool(name="p3s", bufs=4) as p3s,
                tc.tile_pool(name="p3o", bufs=2) as p3o,
                tc.tile_pool(name="p3ps", bufs=1, space="PSUM") as p3ps,
                tc.tile_pool(name="poolps", bufs=1, space="PSUM") as poolps,
            ):
                pool_ps = poolps.tile([NG, 128], dt.float32)
                for j in range(NBLK):
                    osb = p3s.tile([128, NT1], dt.int32, tag="osb", name=f"osb3_{j}")
                    nc.sync.dma_start(osb[:], sidx[j])
                    dl = p3s.tile([128, NT1], dt.float32, tag="dl", name=f"dl3_{j}")
                    nc.sync.dma_start(dl[:], dloc[j])
                    oT = p3g.tile([128, NT1 * 128], dt.bfloat16, tag="oT",
                                  name=f"oT3_{j}")
                    nc.sync.dma_start(oT[:], onehotT[j])
                    adt = p3s.tile([128, 1], dt.bfloat16, tag="adt", name=f"adt3_{j}")
                    nc.sync.dma_start(adt[:],
                                      h2_sh[j * 128:(j + 1) * 128, 129:130])
                    bh = p3s.tile([128, NG], dt.bfloat16, tag="bh", name=f"bh{j}")
                    nc.sync.dma_start(bh[:], bhot[j])
                    G = p3g.tile([128, NT1, ROW2], dt.bfloat16, tag="G2",
                                 name=f"G2_{j}")
                    for t in range(NT1):
                        nc.gpsimd.indirect_dma_start(
                            G[:, t, :], None, h2f[:],
                            bass.IndirectOffsetOnAxis(ap=osb[:, t:t + 1], axis=0))
                    adps = p3ps.tile([128, NT1], dt.float32, tag="adps",
                                     name=f"adps3_{j}")
                    for t in range(NT1):
                        nc.tensor.matmul(adps[:, t:t + 1],
                                         oT[:, t * 128:(t + 1) * 128],
                                         adt[:], start=True, stop=True)
                    ade = p3s.tile([128, NT1], dt.float32, tag="ade",
                                   name=f"ade3_{j}")
                    nc.vector.tensor_copy(ade[:], adps[:])
                    z = p3s.tile([128, NT1], dt.float32, tag="z", name=f"z3_{j}")
                    nc.vector.tensor_tensor(z[:], G[:, :, 128], ade[:], AOT.add)
                    z2 = p3s.tile([128, NT1], dt.float32, tag="z2", name=f"z23_{j}")
                    nc.vector.tensor_scalar(z2[:], z[:], 0.2, None, AOT.mult)
                    nc.vector.tensor_tensor(z[:], z[:], z2[:], AOT.max)
                    e = p3s.tile([128, NT1], dt.float32, tag="e", name=f"e3_{j}")
                    nc.scalar.activation(e[:], z[:], AFT.Exp)
                    accF = p3ps.tile([128, 128], dt.float32, tag="F", name=f"F3_{j}")
                    accD = p3ps.tile([128, 1], dt.float32, tag="Dn", name=f"Dn3_{j}")
                    for t in range(NT1):
                        Sa = p3s.tile([128, 128], dt.bfloat16, tag="Sa",
                                      name=f"Sa3_{j}_{t}")
                        nc.vector.tensor_scalar(
                            Sa[:], iota_sb[:], dl[:, t:t + 1], e[:, t:t + 1],
                            AOT.is_equal, AOT.mult)
                        nc.tensor.matmul(accF[:], Sa[:], G[:, t, 0:128],
                                         start=(t == 0), stop=(t == NT1 - 1))
                        nc.tensor.matmul(accD[:], Sa[:], ones_sb[:],
                                         start=(t == 0), stop=(t == NT1 - 1))
                    den = p3o.tile([128, 1], dt.float32, tag="den", name=f"den3_{j}")
                    nc.vector.tensor_scalar(den[:], accD[:], 1e-6, None, AOT.add)
                    rcp = p3o.tile([128, 1], dt.float32, tag="rcp", name=f"rcp3_{j}")
                    nc.vector.reciprocal(rcp[:], den[:])
                    ov = p3o.tile([128, 128], dt.float32, tag="ov", name=f"ov{j}")
                    nc.vector.tensor_scalar(ov[:], accF[:], rcp[:, 0:1], None,
                                            AOT.mult)
                    nc.vector.tensor_tensor(ov[:], ov[:], b2_sb[:], AOT.add)
                    ob = p3o.tile([128, 128], dt.bfloat16, tag="ob", name=f"ob{j}")
                    nc.vector.tensor_scalar(ob[:], ov[:], 0.0, None, AOT.max)
                    nc.tensor.matmul(pool_ps[:], bh[:], ob[:],
                                     start=(j == 0), stop=(j == NBLK - 1))
                pc = p3o.tile([NG, 128], dt.float32, tag="pc")
                nc.vector.tensor_copy(pc[:], pool_ps[:])
                nc.sync.dma_start(pout[:], pc[:])

    nc.compile()
    return nc


def prepare_inputs(inputs, cfg):
    """Host-side sharding/layout. Returns in_maps (one dict per core)."""
    IN_DIM, HOG = cfg["in_dim"], cfg["hog"]
    N, NPC, NBLK, NT1, NG = cfg["n"], cfg["npc"], cfg["nblk"], cfg["nt1"], cfg["ng"]
    NPAD = NPC * NCORES

    x = np.asarray(inputs["x"], np.float32)
    ei = np.asarray(inputs["edge_index"])
    batch = np.asarray(inputs["batch"]).astype(np.int64)
    W1 = np.asarray(inputs["W1"], np.float32)
    a_src1 = np.asarray(inputs["a_src1"], np.float32)
    a_dst1 = np.asarray(inputs["a_dst1"], np.float32)
    W2 = np.asarray(inputs["W2"], np.float32)
    a_src2 = np.asarray(inputs["a_src2"], np.float32)
    a_dst2 = np.asarray(inputs["a_dst2"], np.float32)
    attn_W = np.asarray(inputs["attn_W"], np.float32)
    attn_b = np.asarray(inputs["attn_b"], np.float32)
    b1 = np.asarray(inputs["b1"], np.float32)
    b2 = np.asarray(inputs["b2"], np.float32)

    # augmented weights
    w1aug = np.zeros((IN_DIM, 262), np.float32)
    w1aug[:, 0:256] = W1
    w1aug[:, 256] = W1[:, 0:128] @ a_src1[0]
    w1aug[:, 257] = W1[:, 128:256] @ a_src1[1]
    w1aug[:, 258] = W1[:, 0:128] @ a_dst1[0]
    w1aug[:, 259] = W1[:, 128:256] @ a_dst1[1]
    w1aug[:, 260:262] = attn_W
    w2aug = np.zeros((256, 132), np.float32)
    w2aug[:, 0:128] = W2
    w2aug[:, 128] = W2 @ a_src2[0]
    w2aug[:, 129] = W2 @ a_dst2[0]

    xT = np.zeros((IN_DIM, NPAD), BF16)
    xT[:, :N] = np.ascontiguousarray(x.T).astype(BF16)

    # edges sorted by destination, self loops appended
    idt = ei.dtype
    src = np.concatenate([ei[0], np.arange(N, dtype=idt)]).astype(np.int64)
    dst = np.concatenate([ei[1], np.arange(N, dtype=idt)]).astype(np.int64)
    order = np.argsort(dst, kind="stable")
    src_s, dst_s = src[order], dst[order]
    nblk_g = NPAD // 128
    L = NT1 * 128
    cnt = np.bincount(dst_s // 128, minlength=nblk_g)
    assert cnt.max() <= L, (cnt.max(), L)
    offs = np.concatenate([[0], np.cumsum(cnt)])
    sidx_all = np.zeros((nblk_g, L), np.int64)
    dloc_all = np.full((nblk_g, L), -1.0, np.float32)
    ohT_all = np.zeros((nblk_g, 128, L), BF16)
    for b in range(nblk_g):
        s, e = offs[b], offs[b + 1]
        n = e - s
        sidx_all[b, :n] = src_s[s:e]
        dl = (dst_s[s:e] - 128 * b).astype(np.int64)
        dloc_all[b, :n] = dl
        ohT_all[b, dl, np.arange(n)] = 1.0

    bh_all = np.zeros((nblk_g, 128, NG), np.float32)
    for b in range(nblk_g):
        base = 128 * b
        hi = min(N - base, 128)
        if hi > 0:
            bh_all[b, np.arange(hi), batch[base:base + hi]] = 1.0

    iota_t = np.tile(np.arange(128, dtype=np.float32), (128, 1)).astype(BF16)
    b1b = np.tile(b1[None, :], (128, 1)).astype(np.float32)
    b2b = np.tile(b2[None, :], (128, 1)).astype(np.float32)
    abb = np.tile(attn_b[None, :], (128, 1)).astype(np.float32)

    in_maps = []
    for c in range(NCORES):
        blo, bhi = c * NBLK, (c + 1) * NBLK
        sidx_c = np.stack([
            np.ascontiguousarray(sidx_all[b].reshape(NT1, 128).T)
            for b in range(blo, bhi)
        ]).astype(np.int32)
        dloc_c = np.stack([
            np.ascontiguousarray(dloc_all[b].reshape(NT1, 128).T)
            for b in range(blo, bhi)
        ])
        in_maps.append({
            "xT": np.ascontiguousarray(xT[:, c * NPC:(c + 1) * NPC]),
            "w1": w1aug.astype(BF16),
            "w2": np.ascontiguousarray(
                w2aug.reshape(2, 128, 132).astype(BF16)),
            "b1b": b1b, "b2b": b2b, "abb": abb, "iota": iota_t,
            "sidx": sidx_c, "dloc": dloc_c,
            "onehotT": ohT_all[blo:bhi],
            "bhot": bh_all[blo:bhi].astype(BF16),
        })
    return in_maps


def _edge_cfg(inputs):
    ei = np.asarray(inputs["edge_index"])
    x = np.asarray(inputs["x"])
    n = x.shape[0]
    npad = -(-n // (128 * NCORES)) * 128 * NCORES
    npc = npad // NCORES
    dst = np.concatenate([ei[1].astype(np.int64), np.arange(n, dtype=np.int64)])
    cnt = np.bincount(dst // 128, minlength=npad // 128)
    nt1 = int(-(-cnt.max() // 128))
    return {
        "n": n, "npc": npc, "nblk": npc // 128, "nt1": nt1,
        "in_dim": x.shape[1], "hog": 4464 if x.shape[1] == 4527 else None,
        "ng": 64,
    }


def kernel(**inputs):
    global LAST_EXEC_NS
    cfg = _edge_cfg(inputs)
    if cfg["hog"] is None:
        raise ValueError("unexpected input width")
    batch = np.asarray(inputs["batch"]).astype(np.int64)
    Wc1 = np.asarray(inputs["Wc1"], np.float32)
    bc1 = np.asarray(inputs["bc1"], np.float32)
    Wc2 = np.asarray(inputs["Wc2"], np.float32)
    bc2 = np.asarray(inputs["bc2"], np.float32)

    in_maps = prepare_inputs(inputs, cfg)
    nc = build_program(cfg)

    profile = os.environ.get("AGAT_PROFILE", "") == "1"
    if profile:
        _install_ntff_hook()
    res = run_bass_kernel_spmd(
        nc, in_maps, core_ids=list(range(NCORES)), trace=profile,
        tmpdir=os.environ.get("AGAT_PROFILE_DIR") or None,
    )
    if profile:
        LAST_EXEC_NS = res.exec_time_ns

    pooled = np.zeros((cfg["ng"], 128), np.float64)
    for c in range(NCORES):
        pooled += res.results[c]["pout"].astype(np.float64)
    cntg = np.bincount(batch, minlength=cfg["ng"]).astype(np.float64)
    pooled = (pooled / np.maximum(cntg, 1.0)[:, None]).astype(np.float32)
    z = np.maximum(pooled @ Wc1 + bc1, 0.0)
    return (z @ Wc2 + bc2).astype(np.float32)
